# revision 7
# baseline (speedup 1.0000x reference)
"""GAT layer (4-head masked attention over an 8192-node graph) on 8 TRN2 NeuronCores.

Sharding: query/node dim N=8192 split across 8 cores (1024 rows each); K/V and
projection weights replicated. Per core the kernel computes, in transposed-score
layout ST[m, n] (partition = key index m, free = query index n):

    QT/KT = W.T @ x.T        (fp32r matmuls, 2 heads packed per 128 partitions)
    V' = x @ Wv + bv         (bf16, with a ones-column per head appended)
    ST = K_h Q_h^T / 8       (fp32r, two 64-contraction row-tiled matmuls)
    P  = exp(ST) * edge      (ACT exp -> bf16, DVE masked multiply; no row-max
                              subtraction needed: scores are O(1) by construction)
    ctxT = V'^T @ P          (bf16; the ones-column makes row 64 the softmax
                              denominator, so no separate reduction is needed)
    out = (ctxT rows 0:64) / (ctxT row 64), PE-transposed back to [n, d].

Host side only reshapes/slices inputs, converts the {0,1} edge mask to bf16,
and concatenates per-core outputs.
"""

import sys

if "/opt/trn_rl_repo" not in sys.path:
    sys.path.insert(0, "/opt/trn_rl_repo")

import numpy as np
import ml_dtypes

import concourse.bass as bass
import concourse.tile as tile
from concourse import mybir
from concourse import bass_utils

N = 8192
D = 256
H = 4
DH = 64
NCORES = 8
NLOC = N // NCORES          # 1024 query rows per core
MT = N // 128               # 64 key tiles
VROW = H * (DH + 1)         # 260: per key-tile V' row: 4 heads x (64 V cols + ones)

F32 = mybir.dt.float32
F32R = mybir.dt.float32r
BF16 = mybir.dt.bfloat16

import os as _os
MAX_DRAIN_WAITS = 1  # HW-tested: 2 waits/inst fails codegen on every encoding here
# headout: head-outer attention order -> only one PV PSUM accumulator live at a
# time (2 banks incl. double-buffer vs 4), freeing PSUM for 1536-wide score
# tiles: exp instruction count drops 256 -> 176 (ACT is the bottleneck engine;
# each instruction carries ~352 cycles of fixed overhead), and the edge-mask
# multiplies widen 512 -> 1536 (DVE instruction count 512 -> 176). Edge tiles
# for the whole 512-query chunk stay resident in SBUF (~36KB/partition) since
# all 4 heads re-read them.
VARIANT = _os.environ.get("KERNEL_VARIANT", "headout")


def _split_drain_waits(nc):
    """walrus in this container rejects >1 sync-wait on several instruction
    encodings (Drain/TPB_CTRL_NO_STRUCT, fp32 matmul/S3_LW_STRUCT, ...).
    Hoist excess waits onto preceding same-engine InstNoOp carriers — the
    engine executes them in order, so semantics are unchanged."""
    for fn in nc.m.functions:
        for bb in fn.blocks:
            new_insts = []
            for inst in bb.instructions:
                si = inst.sync_info
                waits = list(si.on_wait) if si and si.on_wait else []
                if len(waits) > MAX_DRAIN_WAITS:
                    groups = [
                        waits[i : i + MAX_DRAIN_WAITS]
                        for i in range(0, len(waits), MAX_DRAIN_WAITS)
                    ]
                    for g in groups[:-1]:
                        d = mybir.InstNoOp(
                            name=nc.get_next_instruction_name(),
                            ins=[],
                            outs=[],
                        )
                        d.engine = inst.engine
                        d.sync_info = mybir.SyncInfo(on_wait=g, on_update=[])
                        new_insts.append(d)
                    inst.sync_info = mybir.SyncInfo(
                        on_wait=groups[-1], on_update=list(si.on_update)
                    )
                new_insts.append(inst)
            bb.instructions = new_insts


def build_module(split_drains=True, attn_reps=1, attn_loop=None, variant=None):
    var = VARIANT if variant is None else variant
    nc = bass.Bass(
        "TRN2",
        target_bir_lowering=False,
        debug=False,
        enable_asserts=True,
        num_devices=NCORES,
    )

    # Per-core DRAM I/O. xTr[p, j, m] = x[m, 128j+p]; w*[p, j, o] = W[128j+p, o].
    xTr = nc.dram_tensor("xTr", [128, 2, N], F32R, kind="ExternalInput").ap()
    xq = nc.dram_tensor("xq", [128, 2, NLOC], F32R, kind="ExternalInput").ap()
    edge = nc.dram_tensor("edge", [N, NLOC], BF16, kind="ExternalInput").ap()
    wq = nc.dram_tensor("wq", [128, 2, D], F32R, kind="ExternalInput").ap()
    wk = nc.dram_tensor("wk", [128, 2, D], F32R, kind="ExternalInput").ap()
    wv = nc.dram_tensor("wv", [128, 2, D], F32R, kind="ExternalInput").ap()
    bqc = nc.dram_tensor("bqc", [128, 2], F32, kind="ExternalInput").ap()
    bkc = nc.dram_tensor("bkc", [128, 2], F32, kind="ExternalInput").ap()
    bvb = nc.dram_tensor("bvb", [128, D], F32, kind="ExternalInput").ap()
    ident = nc.dram_tensor("ident", [128, 128], F32, kind="ExternalInput").ap()
    identb = nc.dram_tensor("identb", [128, 128], BF16, kind="ExternalInput").ap()
    out = nc.dram_tensor("out", [NLOC, D], F32, kind="ExternalOutput").ap()

    Ident = mybir.ActivationFunctionType.Identity
    Exp = mybir.ActivationFunctionType.Exp

    ho = var == "headout"
    with tile.TileContext(nc) as tc:
        with (
            tc.tile_pool(name="const", bufs=1) as cpool,
            tc.tile_pool(name="big", bufs=1) as bigpool,
            tc.tile_pool(name="xs", bufs=3) as xpool,
            tc.tile_pool(
                name="edge", bufs=(22 if ho else 6 if var == "bufs6" else 4)
            ) as epool,
            tc.tile_pool(name="p", bufs=(6 if ho else 8)) as ppool,
            tc.tile_pool(name="ctx", bufs=2) as ctxpool,
            tc.tile_pool(name="outs", bufs=3) as opool,
            tc.tile_pool(name="rc", bufs=8) as rpool,
            tc.tile_pool(
                name="ps_s", bufs=(1 if var == "batch2" else 2), space="PSUM"
            ) as ps_s,
            tc.tile_pool(name="ps_pv", bufs=(2 if ho else 4), space="PSUM") as ps_pv,
        ):
            # ---- constants ----
            wq_sb = cpool.tile([128, 2, D], F32R, tag="wq")
            nc.sync.dma_start(wq_sb[:], wq[:])
            wk_sb = cpool.tile([128, 2, D], F32R, tag="wk")
            nc.sync.dma_start(wk_sb[:], wk[:])
            wv_sb = cpool.tile([128, 2, D], F32R, tag="wv")
            nc.sync.dma_start(wv_sb[:], wv[:])
            bq_sb = cpool.tile([128, 2], F32, tag="bq")
            nc.sync.dma_start(bq_sb[:], bqc[:])
            bk_sb = cpool.tile([128, 2], F32, tag="bk")
            nc.sync.dma_start(bk_sb[:], bkc[:])
            bv_sb = cpool.tile([128, D], F32, tag="bv")
            nc.sync.dma_start(bv_sb[:], bvb[:])
            id_sb = cpool.tile([128, 128], F32, tag="id")
            nc.sync.dma_start(id_sb[:], ident[:])
            idb_sb = cpool.tile([128, 128], BF16, tag="idb")
            nc.sync.dma_start(idb_sb[:], identb[:])
            xq_sb = cpool.tile([128, 2, NLOC], F32R, tag="xq")
            nc.sync.dma_start(xq_sb[:], xq[:])

            QT = bigpool.tile([128, 2, NLOC], BF16, tag="qt")
            KT = bigpool.tile([128, 2, N], BF16, tag="kt")
            Vp = bigpool.tile([128, MT * VROW], BF16, tag="vp")
            nc.vector.memset(Vp[:], 1.0)

            # ---- Q projection: QT[p, j, n] = sum_d Wq[d, 128j+p] x[n0+n, d] + bq ----
            for j in range(2):
                if ho:
                    q_ps = ps_s.tile([128, 3, 512], F32, tag="sc")
                    qf = q_ps[:, :, :].rearrange("p a b -> p (a b)")
                else:
                    q_ps = ps_s.tile([128, NLOC], F32, tag="sc")
                    qf = q_ps[:]
                for c in range(2):
                    for ji in range(2):
                        nc.tensor.matmul(
                            qf[:, c * 512 : (c + 1) * 512],
                            lhsT=wq_sb[:, ji, j * 128 : (j + 1) * 128],
                            rhs=xq_sb[:, ji, c * 512 : (c + 1) * 512],
                            start=(ji == 0),
                            stop=(ji == 1),
                        )
                nc.scalar.activation(
                    QT[:, j, :], qf[:, 0:NLOC], Ident, bias=bq_sb[:, j : j + 1]
                )

            # ---- K/V projections, streaming x.T in 16 chunks of 512 keys ----
            for mc in range(16):
                xc = xpool.tile([128, 2, 512], F32R, tag="xc")
                nc.sync.dma_start(xc[:], xTr[:, :, mc * 512 : (mc + 1) * 512])
                for j in range(2):
                    k_ps = ps_pv.tile([128, 512], F32, tag="pv")
                    for ji in range(2):
                        nc.tensor.matmul(
                            k_ps[:],
                            lhsT=wk_sb[:, ji, j * 128 : (j + 1) * 128],
                            rhs=xc[:, ji, :],
                            start=(ji == 0),
                            stop=(ji == 1),
                        )
                    nc.scalar.activation(
                        KT[:, j, mc * 512 : (mc + 1) * 512],
                        k_ps[:],
                        Ident,
                        bias=bk_sb[:, j : j + 1],
                    )
                for mt in range(4):
                    t = mc * 4 + mt
                    v_ps = ps_pv.tile([128, D], F32, tag="pv")
                    for ji in range(2):
                        nc.tensor.matmul(
                            v_ps[:],
                            lhsT=xc[:, ji, mt * 128 : (mt + 1) * 128],
                            rhs=wv_sb[:, ji, :],
                            start=(ji == 0),
                            stop=(ji == 1),
                        )
                    out_v = Vp[:, t * VROW : (t + 1) * VROW].rearrange(
                        "p (h q) -> p h q", h=H
                    )[:, :, 0:DH]
                    nc.vector.tensor_add(
                        out_v,
                        v_ps[:].rearrange("p (h q) -> p h q", h=H),
                        bv_sb[:].rearrange("p (h q) -> p h q", h=H),
                    )

            def emit_attention():
                # ---- attention over the core's NLOC queries, in two 512 chunks ----
                # attn_reps>1 repeats the (idempotent) attention phase for timing.
                for rep in range(attn_reps):
                  for c in range(2):
                    n0 = c * 512
                    pv_ps = [
                        ps_pv.tile([128, 512], F32, tag="pv", name=f"pv_{rep}_{c}_{h}")
                        for h in range(H)
                    ]
                    if VARIANT == "batch2":
                        # 2 m-tiles per step: one [128,2048] score tile (4 PSUM
                        # banks, single-buffered) and one exp per (pair, 2 tiles)
                        # halves the ACT per-instruction overhead.
                        for tb in range(MT // 2):
                            e2 = epool.tile([128, 2, 512], BF16, tag="e")
                            nc.sync.dma_start(
                                e2[:],
                                edge[
                                    tb * 256 : (tb + 1) * 256, n0 : n0 + 512
                                ].rearrange("(tt p) q -> p tt q", p=128),
                            )
                            for pair in range(2):
                                s_ps = ps_s.tile([128, 2048], F32, tag="sc")
                                for i in range(2):
                                    po = i * 64
                                    for tt in range(2):
                                        t = 2 * tb + tt
                                        nc.tensor.matmul(
                                            s_ps[
                                                :,
                                                i * 1024
                                                + tt * 512 : i * 1024
                                                + (tt + 1) * 512,
                                            ],
                                            lhsT=KT[
                                                po : po + 64,
                                                pair,
                                                t * 128 : (t + 1) * 128,
                                            ],
                                            rhs=QT[
                                                po : po + 64, pair, n0 : n0 + 512
                                            ],
                                            start=True,
                                            stop=True,
                                        )
                                p_sb = ppool.tile([128, 2048], BF16, tag="p")
                                nc.scalar.activation(p_sb[:], s_ps[:], Exp, scale=0.125)
                                for i in range(2):
                                    for tt in range(2):
                                        nc.vector.tensor_mul(
                                            p_sb[
                                                :,
                                                i * 1024
                                                + tt * 512 : i * 1024
                                                + (tt + 1) * 512,
                                            ],
                                            p_sb[
                                                :,
                                                i * 1024
                                                + tt * 512 : i * 1024
                                                + (tt + 1) * 512,
                                            ],
                                            e2[:, tt, :],
                                        )
                                for i in range(2):
                                    h = 2 * pair + i
                                    for tt in range(2):
                                        t = 2 * tb + tt
                                        nc.tensor.matmul(
                                            pv_ps[h][0:65, :],
                                            lhsT=Vp[
                                                :,
                                                t * VROW
                                                + h * 65 : t * VROW
                                                + h * 65
                                                + 65,
                                            ],
                                            rhs=p_sb[
                                                :,
                                                i * 1024
                                                + tt * 512 : i * 1024
                                                + (tt + 1) * 512,
                                            ],
                                            start=(t == 0),
                                            stop=(t == MT - 1),
                                        )
                    for t in range(MT if VARIANT != "batch2" else 0):
                        e_sb = epool.tile([128, 512], BF16, tag="e")
                        if VARIANT != "nodma":
                            nc.sync.dma_start(
                                e_sb[:], edge[t * 128 : (t + 1) * 128, n0 : n0 + 512]
                            )
                        for pair in range(2):
                            s_ps = ps_s.tile([128, 1024], F32, tag="sc")
                            pe_mask = VARIANT == "pemask"
                            for i in range(2 if VARIANT != "noqk" else 0):
                                po = i * 64
                                nc.tensor.matmul(
                                    s_ps[:, i * 512 : (i + 1) * 512],
                                    lhsT=KT[
                                        po : po + 64, pair, t * 128 : (t + 1) * 128
                                    ],
                                    rhs=QT[po : po + 64, pair, n0 : n0 + 512],
                                    start=True,
                                    stop=not pe_mask,
                                )
                            if pe_mask:
                                # accumulate the additive -inf mask via the PE:
                                # s_ps[m, n] += sum_k I[k, m] * M[k, n] = M[m, n]
                                for i in range(2):
                                    nc.tensor.matmul(
                                        s_ps[:, i * 512 : (i + 1) * 512],
                                        lhsT=idb_sb[:],
                                        rhs=e_sb[:],
                                        start=False,
                                        stop=True,
                                    )
                            p_sb = ppool.tile([128, 1024], BF16, tag="p")
                            if VARIANT == "noexp":
                                nc.vector.tensor_copy(p_sb[:], s_ps[:])
                            else:
                                nc.scalar.activation(
                                    p_sb[:], s_ps[:], Exp, scale=0.125
                                )
                            if VARIANT in ("nomask", "pemask"):
                                pass
                            else:
                                # two plain [128, 512] muls: a broadcast in1 AP
                                # degrades the DVE perf-mode (HW-measured +19us
                                # per pass vs this form)
                                for i in range(2):
                                    nc.vector.tensor_mul(
                                        p_sb[:, i * 512 : (i + 1) * 512],
                                        p_sb[:, i * 512 : (i + 1) * 512],
                                        e_sb[:],
                                    )
                            for i in range(2 if VARIANT != "nopv" else 0):
                                h = 2 * pair + i
                                nc.tensor.matmul(
                                    pv_ps[h][0:65, :],
                                    lhsT=Vp[
                                        :, t * VROW + h * 65 : t * VROW + h * 65 + 65
                                    ],
                                    rhs=p_sb[:, i * 512 : (i + 1) * 512],
                                    start=(t == 0),
                                    stop=(t == MT - 1),
                                )

                    # epilogue: divide by denominator row, transpose to [n, d]
                    ctx = ctxpool.tile([128, H * 512], F32, tag="ctx")
                    for h in range(H):
                        nc.vector.tensor_copy(
                            ctx[0:65, h * 512 : (h + 1) * 512], pv_ps[h][0:65, :]
                        )
                    for sub in range(4):
                        tr_ps = ps_s.tile([128, 1024], F32, tag="sc")
                        for h in range(H):
                            nc.tensor.transpose(
                                tr_ps[:, h * 65 : h * 65 + 65],
                                ctx[0:65, h * 512 + sub * 128 : h * 512 + (sub + 1) * 128],
                                id_sb[0:65, 0:65],
                            )
                        o_sb = opool.tile([128, D], F32, tag="o")
                        for h in range(H):
                            rc = rpool.tile([128, 1], F32, tag="rc")
                            nc.vector.reciprocal(
                                rc[:], tr_ps[:, h * 65 + 64 : h * 65 + 65]
                            )
                            nc.vector.tensor_scalar_mul(
                                o_sb[:, h * DH : (h + 1) * DH],
                                tr_ps[:, h * 65 : h * 65 + 64],
                                rc[:],
                            )
                        nc.sync.dma_start(
                            out[n0 + sub * 128 : n0 + (sub + 1) * 128, :], o_sb[:]
                        )

            def emit_attention_headout():
                # head-outer: one head's full key sweep at a time. Only one PV
                # accumulator bank is live (+1 for overlap), so score tiles can
                # span 3 key-tiles (1536 fp32 = 3 PSUM banks, double-buffered):
                # 176 exp instructions/pass instead of 256, 176 mask-muls
                # instead of 512. Edge tiles are DMA'd once per chunk (h==0)
                # and re-read by the other 3 heads from SBUF.
                G = (MT + 2) // 3  # 22 tiles of up to 3 key-tiles each
                for rep in range(attn_reps):
                  for c in range(2):
                    n0 = c * 512
                    ctx = ctxpool.tile([128, H * 512], F32, tag="ctx")
                    etiles = []
                    for h in range(H):
                        pair, i = divmod(h, 2)
                        po = i * 64
                        pv = ps_pv.tile(
                            [128, 512], F32, tag="pv", name=f"pv_{rep}_{c}_{h}"
                        )
                        for g in range(G):
                            t0 = 3 * g
                            tt = min(3, MT - t0)
                            if h == 0:
                                e3 = epool.tile([128, 3, 512], BF16, tag="e")
                                nc.sync.dma_start(
                                    e3[:, 0:tt, :],
                                    edge[
                                        t0 * 128 : (t0 + tt) * 128, n0 : n0 + 512
                                    ].rearrange("(tt p) q -> p tt q", p=128),
                                )
                                etiles.append(e3)
                            else:
                                e3 = etiles[g]
                            s_ps = ps_s.tile([128, 3, 512], F32, tag="sc")
                            for k in range(tt):
                                t = t0 + k
                                nc.tensor.matmul(
                                    s_ps[:, k, :],
                                    lhsT=KT[
                                        po : po + 64, pair, t * 128 : (t + 1) * 128
                                    ],
                                    rhs=QT[po : po + 64, pair, n0 : n0 + 512],
                                    start=True,
                                    stop=True,
                                )
                            p_sb = ppool.tile([128, 3, 512], BF16, tag="p")
                            nc.scalar.activation(
                                p_sb[:, 0:tt, :], s_ps[:, 0:tt, :], Exp, scale=0.125
                            )
                            nc.vector.tensor_mul(
                                p_sb[:, 0:tt, :], p_sb[:, 0:tt, :], e3[:, 0:tt, :]
                            )
                            for k in range(tt):
                                t = t0 + k
                                nc.tensor.matmul(
                                    pv[0:65, :],
                                    lhsT=Vp[
                                        :, t * VROW + h * 65 : t * VROW + h * 65 + 65
                                    ],
                                    rhs=p_sb[:, k, :],
                                    start=(t == 0),
                                    stop=(t == MT - 1),
                                )
                        nc.vector.tensor_copy(
                            ctx[0:65, h * 512 : (h + 1) * 512], pv[0:65, :]
                        )
                    # epilogue: divide by denominator row, transpose to [n, d]
                    for sub in range(4):
                        tr_ps = ps_s.tile([128, 3, 512], F32, tag="sc")
                        trf = tr_ps[:, :, :].rearrange("p a b -> p (a b)")
                        for h in range(H):
                            nc.tensor.transpose(
                                trf[:, h * 65 : h * 65 + 65],
                                ctx[
                                    0:65,
                                    h * 512 + sub * 128 : h * 512 + (sub + 1) * 128,
                                ],
                                id_sb[0:65, 0:65],
                            )
                        o_sb = opool.tile([128, D], F32, tag="o")
                        for h in range(H):
                            rc = rpool.tile([128, 1], F32, tag="rc")
                            nc.vector.reciprocal(
                                rc[:], trf[:, h * 65 + 64 : h * 65 + 65]
                            )
                            nc.vector.tensor_scalar_mul(
                                o_sb[:, h * DH : (h + 1) * DH],
                                trf[:, h * 65 : h * 65 + 64],
                                rc[:],
                            )
                        nc.sync.dma_start(
                            out[n0 + sub * 128 : n0 + (sub + 1) * 128, :], o_sb[:]
                        )

            emit = emit_attention_headout if ho else emit_attention
            if attn_loop is None:
                emit()
            else:
                with tc.For_i(0, attn_loop, 1):
                    emit()

    if split_drains:
        _split_drain_waits(nc)
    return nc


def prep_in_maps(x, edge, Wq, bq, Wk, bk, Wv, bv):
    bf16 = ml_dtypes.bfloat16
    x = np.ascontiguousarray(np.asarray(x, np.float32))
    edge = np.asarray(edge)
    xTr = np.ascontiguousarray(x.T.reshape(2, 128, N).transpose(1, 0, 2))

    def wprep(W):
        return np.ascontiguousarray(
            np.asarray(W, np.float32).reshape(2, 128, D).transpose(1, 0, 2)
        )

    def bprep(b):
        return np.ascontiguousarray(np.asarray(b, np.float32).reshape(2, 128).T)

    common = {
        "xTr": xTr,
        "wq": wprep(Wq),
        "wk": wprep(Wk),
        "wv": wprep(Wv),
        "bqc": bprep(bq),
        "bkc": bprep(bk),
        "bvb": np.ascontiguousarray(
            np.broadcast_to(np.asarray(bv, np.float32), (128, D))
        ),
        "ident": np.eye(128, dtype=np.float32),
        "identb": np.eye(128, dtype=np.float32).astype(bf16),
    }
    if VARIANT == "pemask":
        # additive mask: 0 where edge==1, -80000 (pre-exp-scale) where edge==0
        edge_bf = ((edge.astype(np.float32) - 1.0) * 80000.0).astype(bf16)
    else:
        edge_bf = edge.astype(bf16)
    in_maps = []
    for core in range(NCORES):
        n0 = core * NLOC
        m = dict(common)
        m["xq"] = np.ascontiguousarray(xTr[:, :, n0 : n0 + NLOC])
        m["edge"] = np.ascontiguousarray(edge_bf[:, n0 : n0 + NLOC])
        in_maps.append(m)
    return in_maps


_CACHED_NC = None


def kernel(x, edge, Wq, bq, Wk, bk, Wv, bv):
    global _CACHED_NC
    if _CACHED_NC is None:
        _CACHED_NC = build_module()
    nc = _CACHED_NC
    in_maps = prep_in_maps(x, edge, Wq, bq, Wk, bk, Wv, bv)
    res = bass_utils.run_bass_kernel_spmd(nc, in_maps, core_ids=list(range(NCORES)))
    out = np.concatenate([r["out"] for r in res.results], axis=0)
    return out.astype(np.float32)


if __name__ == "__main__":
    rng = np.random.default_rng(0)
    x = rng.standard_normal((N, D), dtype=np.float32)
    edge = rng.integers(0, 2, size=(N, N)).astype(np.int32)
    mk = lambda *s: (rng.standard_normal(s, dtype=np.float32) / 16.0)
    o = kernel(
        x, edge, mk(D, D), mk(D) * 0.16, mk(D, D), mk(D) * 0.16, mk(D, D), mk(D) * 0.16
    )
    print(o.shape, o.dtype)



# revision 10
# speedup vs baseline: 1.4325x; 1.4325x over previous
"""GAT layer (4-head masked attention over an 8192-node graph) on 8 TRN2 NeuronCores.

Sharding: query/node dim N=8192 split across 8 cores (1024 rows each); K/V and
projection weights replicated. Per core the kernel computes, in transposed-score
layout ST[m, n] (partition = key index m, free = query index n):

    QT/KT = W.T @ x.T        (fp32r matmuls, 2 heads packed per 128 partitions)
    V' = x @ Wv + bv         (bf16, with a ones-column per head appended)
    ST = K_h Q_h^T / 8       (fp32r, two 64-contraction row-tiled matmuls)
    P  = exp(ST) * edge      (ACT exp -> bf16, DVE masked multiply; no row-max
                              subtraction needed: scores are O(1) by construction)
    ctxT = V'^T @ P          (bf16; the ones-column makes row 64 the softmax
                              denominator, so no separate reduction is needed)
    out = (ctxT rows 0:64) / (ctxT row 64), PE-transposed back to [n, d].

Host side only reshapes/slices inputs, converts the {0,1} edge mask to bf16,
and concatenates per-core outputs.
"""

import sys

if "/opt/trn_rl_repo" not in sys.path:
    sys.path.insert(0, "/opt/trn_rl_repo")

import numpy as np
import ml_dtypes

import concourse.bass as bass
import concourse.tile as tile
from concourse import mybir
from concourse import bass_utils

N = 8192
D = 256
H = 4
DH = 64
NCORES = 8
NLOC = N // NCORES          # 1024 query rows per core
MT = N // 128               # 64 key tiles
VROW = H * (DH + 1)         # 260: per key-tile V' row: 4 heads x (64 V cols + ones)

F32 = mybir.dt.float32
F32R = mybir.dt.float32r
BF16 = mybir.dt.bfloat16

import os as _os
MAX_DRAIN_WAITS = 1  # HW-tested: 2 waits/inst fails codegen on every encoding here
# headout (kept for reference, NOT default): head-outer order frees PSUM for
# 1536-wide score tiles (176 exp/mask instructions instead of 256/512).
# HW-tested 2026-08-09: LOSES ~40% (434us vs 296us interleaved A/B) — ACT
# throughput from PSUM degrades with instruction width (~1.0 cyc/elem at
# 1024-wide, ~1.36 at 2048, ~1.7 at 1536 measured three ways), so the
# overhead saving is swamped. 1024-wide exp is the HW sweet spot; PV
# software-pipelining and flat-2D APs (both tried) don't close the gap.
VARIANT = _os.environ.get("KERNEL_VARIANT", "base")


def _split_drain_waits(nc):
    """walrus in this container rejects >1 sync-wait on several instruction
    encodings (Drain/TPB_CTRL_NO_STRUCT, fp32 matmul/S3_LW_STRUCT, ...).
    Hoist excess waits onto preceding same-engine InstNoOp carriers — the
    engine executes them in order, so semantics are unchanged."""
    for fn in nc.m.functions:
        for bb in fn.blocks:
            new_insts = []
            for inst in bb.instructions:
                si = inst.sync_info
                waits = list(si.on_wait) if si and si.on_wait else []
                if len(waits) > MAX_DRAIN_WAITS:
                    groups = [
                        waits[i : i + MAX_DRAIN_WAITS]
                        for i in range(0, len(waits), MAX_DRAIN_WAITS)
                    ]
                    for g in groups[:-1]:
                        d = mybir.InstNoOp(
                            name=nc.get_next_instruction_name(),
                            ins=[],
                            outs=[],
                        )
                        d.engine = inst.engine
                        d.sync_info = mybir.SyncInfo(on_wait=g, on_update=[])
                        new_insts.append(d)
                    inst.sync_info = mybir.SyncInfo(
                        on_wait=groups[-1], on_update=list(si.on_update)
                    )
                new_insts.append(inst)
            bb.instructions = new_insts


def build_module(split_drains=True, attn_reps=1, attn_loop=None, variant=None):
    var = VARIANT if variant is None else variant
    nc = bass.Bass(
        "TRN2",
        target_bir_lowering=False,
        debug=False,
        enable_asserts=True,
        num_devices=NCORES,
    )

    # Per-core DRAM I/O. xTr[p, j, m] = x[m, 128j+p]; w*[p, j, o] = W[128j+p, o].
    xTr = nc.dram_tensor("xTr", [128, 2, N], F32R, kind="ExternalInput").ap()
    xq = nc.dram_tensor("xq", [128, 2, NLOC], F32R, kind="ExternalInput").ap()
    edge = nc.dram_tensor("edge", [N, NLOC], BF16, kind="ExternalInput").ap()
    wq = nc.dram_tensor("wq", [128, 2, D], F32R, kind="ExternalInput").ap()
    wk = nc.dram_tensor("wk", [128, 2, D], F32R, kind="ExternalInput").ap()
    wv = nc.dram_tensor("wv", [128, 2, D], F32R, kind="ExternalInput").ap()
    bqc = nc.dram_tensor("bqc", [128, 2], F32, kind="ExternalInput").ap()
    bkc = nc.dram_tensor("bkc", [128, 2], F32, kind="ExternalInput").ap()
    bvb = nc.dram_tensor("bvb", [128, D], F32, kind="ExternalInput").ap()
    ident = nc.dram_tensor("ident", [128, 128], F32, kind="ExternalInput").ap()
    identb = nc.dram_tensor("identb", [128, 128], BF16, kind="ExternalInput").ap()
    out = nc.dram_tensor("out", [NLOC, D], F32, kind="ExternalOutput").ap()

    Ident = mybir.ActivationFunctionType.Identity
    Exp = mybir.ActivationFunctionType.Exp

    ho = var == "headout"
    with tile.TileContext(nc) as tc:
        with (
            tc.tile_pool(name="const", bufs=1) as cpool,
            tc.tile_pool(name="big", bufs=1) as bigpool,
            tc.tile_pool(name="xs", bufs=3) as xpool,
            tc.tile_pool(
                name="edge", bufs=(22 if ho else 6 if var == "bufs6" else 4)
            ) as epool,
            tc.tile_pool(name="p", bufs=(6 if ho else 8)) as ppool,
            tc.tile_pool(name="ctx", bufs=2) as ctxpool,
            tc.tile_pool(name="outs", bufs=3) as opool,
            tc.tile_pool(name="rc", bufs=8) as rpool,
            tc.tile_pool(
                name="ps_s", bufs=(1 if var == "batch2" else 2), space="PSUM"
            ) as ps_s,
            tc.tile_pool(name="ps_pv", bufs=(2 if ho else 4), space="PSUM") as ps_pv,
        ):
            # ---- constants ----
            wq_sb = cpool.tile([128, 2, D], F32R, tag="wq")
            nc.sync.dma_start(wq_sb[:], wq[:])
            wk_sb = cpool.tile([128, 2, D], F32R, tag="wk")
            nc.sync.dma_start(wk_sb[:], wk[:])
            wv_sb = cpool.tile([128, 2, D], F32R, tag="wv")
            nc.sync.dma_start(wv_sb[:], wv[:])
            bq_sb = cpool.tile([128, 2], F32, tag="bq")
            nc.sync.dma_start(bq_sb[:], bqc[:])
            bk_sb = cpool.tile([128, 2], F32, tag="bk")
            nc.sync.dma_start(bk_sb[:], bkc[:])
            bv_sb = cpool.tile([128, D], F32, tag="bv")
            nc.sync.dma_start(bv_sb[:], bvb[:])
            id_sb = cpool.tile([128, 128], F32, tag="id")
            nc.sync.dma_start(id_sb[:], ident[:])
            idb_sb = cpool.tile([128, 128], BF16, tag="idb")
            nc.sync.dma_start(idb_sb[:], identb[:])
            xq_sb = cpool.tile([128, 2, NLOC], F32R, tag="xq")
            nc.sync.dma_start(xq_sb[:], xq[:])

            QT = bigpool.tile([128, 2, NLOC], BF16, tag="qt")
            KT = bigpool.tile([128, 2, N], BF16, tag="kt")
            Vp = bigpool.tile([128, MT * VROW], BF16, tag="vp")
            nc.vector.memset(Vp[:], 1.0)

            # ---- Q projection: QT[p, j, n] = sum_d Wq[d, 128j+p] x[n0+n, d] + bq ----
            for j in range(2):
                if ho:
                    q_ps = ps_s.tile([128, 3, 512], F32, tag="sc")
                    qf = q_ps[:, :, :].rearrange("p a b -> p (a b)")
                else:
                    q_ps = ps_s.tile([128, NLOC], F32, tag="sc")
                    qf = q_ps[:]
                for c in range(2):
                    for ji in range(2):
                        nc.tensor.matmul(
                            qf[:, c * 512 : (c + 1) * 512],
                            lhsT=wq_sb[:, ji, j * 128 : (j + 1) * 128],
                            rhs=xq_sb[:, ji, c * 512 : (c + 1) * 512],
                            start=(ji == 0),
                            stop=(ji == 1),
                        )
                nc.scalar.activation(
                    QT[:, j, :], qf[:, 0:NLOC], Ident, bias=bq_sb[:, j : j + 1]
                )

            # ---- K/V projections, streaming x.T in 16 chunks of 512 keys ----
            for mc in range(16):
                xc = xpool.tile([128, 2, 512], F32R, tag="xc")
                nc.sync.dma_start(xc[:], xTr[:, :, mc * 512 : (mc + 1) * 512])
                for j in range(2):
                    k_ps = ps_pv.tile([128, 512], F32, tag="pv")
                    for ji in range(2):
                        nc.tensor.matmul(
                            k_ps[:],
                            lhsT=wk_sb[:, ji, j * 128 : (j + 1) * 128],
                            rhs=xc[:, ji, :],
                            start=(ji == 0),
                            stop=(ji == 1),
                        )
                    nc.scalar.activation(
                        KT[:, j, mc * 512 : (mc + 1) * 512],
                        k_ps[:],
                        Ident,
                        bias=bk_sb[:, j : j + 1],
                    )
                for mt in range(4):
                    t = mc * 4 + mt
                    v_ps = ps_pv.tile([128, D], F32, tag="pv")
                    for ji in range(2):
                        nc.tensor.matmul(
                            v_ps[:],
                            lhsT=xc[:, ji, mt * 128 : (mt + 1) * 128],
                            rhs=wv_sb[:, ji, :],
                            start=(ji == 0),
                            stop=(ji == 1),
                        )
                    out_v = Vp[:, t * VROW : (t + 1) * VROW].rearrange(
                        "p (h q) -> p h q", h=H
                    )[:, :, 0:DH]
                    nc.vector.tensor_add(
                        out_v,
                        v_ps[:].rearrange("p (h q) -> p h q", h=H),
                        bv_sb[:].rearrange("p (h q) -> p h q", h=H),
                    )

            def emit_attention():
                # ---- attention over the core's NLOC queries, in two 512 chunks ----
                # attn_reps>1 repeats the (idempotent) attention phase for timing.
                for rep in range(attn_reps):
                  for c in range(2):
                    n0 = c * 512
                    pv_ps = [
                        ps_pv.tile([128, 512], F32, tag="pv", name=f"pv_{rep}_{c}_{h}")
                        for h in range(H)
                    ]
                    if VARIANT == "batch2":
                        # 2 m-tiles per step: one [128,2048] score tile (4 PSUM
                        # banks, single-buffered) and one exp per (pair, 2 tiles)
                        # halves the ACT per-instruction overhead.
                        for tb in range(MT // 2):
                            e2 = epool.tile([128, 2, 512], BF16, tag="e")
                            nc.sync.dma_start(
                                e2[:],
                                edge[
                                    tb * 256 : (tb + 1) * 256, n0 : n0 + 512
                                ].rearrange("(tt p) q -> p tt q", p=128),
                            )
                            for pair in range(2):
                                s_ps = ps_s.tile([128, 2048], F32, tag="sc")
                                for i in range(2):
                                    po = i * 64
                                    for tt in range(2):
                                        t = 2 * tb + tt
                                        nc.tensor.matmul(
                                            s_ps[
                                                :,
                                                i * 1024
                                                + tt * 512 : i * 1024
                                                + (tt + 1) * 512,
                                            ],
                                            lhsT=KT[
                                                po : po + 64,
                                                pair,
                                                t * 128 : (t + 1) * 128,
                                            ],
                                            rhs=QT[
                                                po : po + 64, pair, n0 : n0 + 512
                                            ],
                                            start=True,
                                            stop=True,
                                        )
                                p_sb = ppool.tile([128, 2048], BF16, tag="p")
                                nc.scalar.activation(p_sb[:], s_ps[:], Exp, scale=0.125)
                                for i in range(2):
                                    for tt in range(2):
                                        nc.vector.tensor_mul(
                                            p_sb[
                                                :,
                                                i * 1024
                                                + tt * 512 : i * 1024
                                                + (tt + 1) * 512,
                                            ],
                                            p_sb[
                                                :,
                                                i * 1024
                                                + tt * 512 : i * 1024
                                                + (tt + 1) * 512,
                                            ],
                                            e2[:, tt, :],
                                        )
                                for i in range(2):
                                    h = 2 * pair + i
                                    for tt in range(2):
                                        t = 2 * tb + tt
                                        nc.tensor.matmul(
                                            pv_ps[h][0:65, :],
                                            lhsT=Vp[
                                                :,
                                                t * VROW
                                                + h * 65 : t * VROW
                                                + h * 65
                                                + 65,
                                            ],
                                            rhs=p_sb[
                                                :,
                                                i * 1024
                                                + tt * 512 : i * 1024
                                                + (tt + 1) * 512,
                                            ],
                                            start=(t == 0),
                                            stop=(t == MT - 1),
                                        )
                    for t in range(MT if VARIANT != "batch2" else 0):
                        e_sb = epool.tile([128, 512], BF16, tag="e")
                        if VARIANT != "nodma":
                            nc.sync.dma_start(
                                e_sb[:], edge[t * 128 : (t + 1) * 128, n0 : n0 + 512]
                            )
                        for pair in range(2):
                            s_ps = ps_s.tile([128, 1024], F32, tag="sc")
                            pe_mask = VARIANT == "pemask"
                            for i in range(2 if VARIANT != "noqk" else 0):
                                po = i * 64
                                nc.tensor.matmul(
                                    s_ps[:, i * 512 : (i + 1) * 512],
                                    lhsT=KT[
                                        po : po + 64, pair, t * 128 : (t + 1) * 128
                                    ],
                                    rhs=QT[po : po + 64, pair, n0 : n0 + 512],
                                    start=True,
                                    stop=not pe_mask,
                                )
                            if pe_mask:
                                # accumulate the additive -inf mask via the PE:
                                # s_ps[m, n] += sum_k I[k, m] * M[k, n] = M[m, n]
                                for i in range(2):
                                    nc.tensor.matmul(
                                        s_ps[:, i * 512 : (i + 1) * 512],
                                        lhsT=idb_sb[:],
                                        rhs=e_sb[:],
                                        start=False,
                                        stop=True,
                                    )
                            p_sb = ppool.tile([128, 1024], BF16, tag="p")
                            if VARIANT == "noexp":
                                nc.vector.tensor_copy(p_sb[:], s_ps[:])
                            else:
                                nc.scalar.activation(
                                    p_sb[:], s_ps[:], Exp, scale=0.125
                                )
                            if VARIANT in ("nomask", "pemask"):
                                pass
                            else:
                                # two plain [128, 512] muls: a broadcast in1 AP
                                # degrades the DVE perf-mode (HW-measured +19us
                                # per pass vs this form)
                                for i in range(2):
                                    nc.vector.tensor_mul(
                                        p_sb[:, i * 512 : (i + 1) * 512],
                                        p_sb[:, i * 512 : (i + 1) * 512],
                                        e_sb[:],
                                    )
                            for i in range(2 if VARIANT != "nopv" else 0):
                                h = 2 * pair + i
                                nc.tensor.matmul(
                                    pv_ps[h][0:65, :],
                                    lhsT=Vp[
                                        :, t * VROW + h * 65 : t * VROW + h * 65 + 65
                                    ],
                                    rhs=p_sb[:, i * 512 : (i + 1) * 512],
                                    start=(t == 0),
                                    stop=(t == MT - 1),
                                )

                    # epilogue: divide by denominator row, transpose to [n, d]
                    ctx = ctxpool.tile([128, H * 512], F32, tag="ctx")
                    for h in range(H):
                        nc.vector.tensor_copy(
                            ctx[0:65, h * 512 : (h + 1) * 512], pv_ps[h][0:65, :]
                        )
                    for sub in range(4):
                        tr_ps = ps_s.tile([128, 1024], F32, tag="sc")
                        for h in range(H):
                            nc.tensor.transpose(
                                tr_ps[:, h * 65 : h * 65 + 65],
                                ctx[0:65, h * 512 + sub * 128 : h * 512 + (sub + 1) * 128],
                                id_sb[0:65, 0:65],
                            )
                        o_sb = opool.tile([128, D], F32, tag="o")
                        for h in range(H):
                            rc = rpool.tile([128, 1], F32, tag="rc")
                            nc.vector.reciprocal(
                                rc[:], tr_ps[:, h * 65 + 64 : h * 65 + 65]
                            )
                            nc.vector.tensor_scalar_mul(
                                o_sb[:, h * DH : (h + 1) * DH],
                                tr_ps[:, h * 65 : h * 65 + 64],
                                rc[:],
                            )
                        nc.sync.dma_start(
                            out[n0 + sub * 128 : n0 + (sub + 1) * 128, :], o_sb[:]
                        )

            def emit_attention_headout():
                # head-outer: one head's full key sweep at a time. Only one PV
                # accumulator bank is live (+1 for overlap), so score tiles can
                # span 3 key-tiles (1536 fp32 = 3 PSUM banks, double-buffered):
                # 176 exp instructions/pass instead of 256, 176 mask-muls
                # instead of 512. Edge tiles are DMA'd once per chunk (h==0)
                # and re-read by the other 3 heads from SBUF.
                G = (MT + 2) // 3  # 22 tiles of up to 3 key-tiles each
                for rep in range(attn_reps):
                  for c in range(2):
                    n0 = c * 512
                    ctx = ctxpool.tile([128, H * 512], F32, tag="ctx")
                    # hoist the chunk's edge DMAs so h==0 mask-muls never wait
                    # on DMA issue order
                    etiles = []
                    for g in range(G):
                        t0 = 3 * g
                        tt = min(3, MT - t0)
                        e3 = epool.tile([128, 3, 512], BF16, tag="e")
                        nc.sync.dma_start(
                            e3[:, 0:tt, :],
                            edge[
                                t0 * 128 : (t0 + tt) * 128, n0 : n0 + 512
                            ].rearrange("(tt p) q -> p tt q", p=128),
                        )
                        etiles.append(e3)
                    for h in range(H):
                        pair, i = divmod(h, 2)
                        po = i * 64
                        pv = ps_pv.tile(
                            [128, 512], F32, tag="pv", name=f"pv_{rep}_{c}_{h}"
                        )

                        def emit_pv(t0, tt, p_sb):
                            for k in range(tt):
                                t = t0 + k
                                nc.tensor.matmul(
                                    pv[0:65, :],
                                    lhsT=Vp[
                                        :, t * VROW + h * 65 : t * VROW + h * 65 + 65
                                    ],
                                    rhs=p_sb[:, k, :],
                                    start=(t == 0),
                                    stop=(t == MT - 1),
                                )

                        # PV runs one tile behind QK/exp/mask: its mask is long
                        # done by then, so no PE instruction ever parks on a
                        # cross-engine wait (PE wait-queue depth is 4).
                        prev = None
                        for g in range(G):
                            t0 = 3 * g
                            tt = min(3, MT - t0)
                            e3 = etiles[g]
                            s_ps = ps_s.tile([128, 3, 512], F32, tag="sc")
                            for k in range(tt):
                                t = t0 + k
                                nc.tensor.matmul(
                                    s_ps[:, k, :],
                                    lhsT=KT[
                                        po : po + 64, pair, t * 128 : (t + 1) * 128
                                    ],
                                    rhs=QT[po : po + 64, pair, n0 : n0 + 512],
                                    start=True,
                                    stop=True,
                                )
                            p_sb = ppool.tile([128, 3, 512], BF16, tag="p")
                            # flatten to 2-D contiguous APs: engines only hit
                            # their fast path / documented rate on flat APs
                            pf = p_sb[:, 0:tt, :].rearrange("p a b -> p (a b)")
                            sf = s_ps[:, 0:tt, :].rearrange("p a b -> p (a b)")
                            ef = e3[:, 0:tt, :].rearrange("p a b -> p (a b)")
                            nc.scalar.activation(pf, sf, Exp, scale=0.125)
                            nc.vector.tensor_mul(pf, pf, ef)
                            if prev is not None:
                                emit_pv(*prev)
                            prev = (t0, tt, p_sb)
                        emit_pv(*prev)
                        nc.vector.tensor_copy(
                            ctx[0:65, h * 512 : (h + 1) * 512], pv[0:65, :]
                        )
                    # epilogue: divide by denominator row, transpose to [n, d]
                    for sub in range(4):
                        tr_ps = ps_s.tile([128, 3, 512], F32, tag="sc")
                        trf = tr_ps[:, :, :].rearrange("p a b -> p (a b)")
                        for h in range(H):
                            nc.tensor.transpose(
                                trf[:, h * 65 : h * 65 + 65],
                                ctx[
                                    0:65,
                                    h * 512 + sub * 128 : h * 512 + (sub + 1) * 128,
                                ],
                                id_sb[0:65, 0:65],
                            )
                        o_sb = opool.tile([128, D], F32, tag="o")
                        for h in range(H):
                            rc = rpool.tile([128, 1], F32, tag="rc")
                            nc.vector.reciprocal(
                                rc[:], trf[:, h * 65 + 64 : h * 65 + 65]
                            )
                            nc.vector.tensor_scalar_mul(
                                o_sb[:, h * DH : (h + 1) * DH],
                                trf[:, h * 65 : h * 65 + 64],
                                rc[:],
                            )
                        nc.sync.dma_start(
                            out[n0 + sub * 128 : n0 + (sub + 1) * 128, :], o_sb[:]
                        )

            emit = emit_attention_headout if ho else emit_attention
            if attn_loop is None:
                emit()
            else:
                with tc.For_i(0, attn_loop, 1):
                    emit()

    if split_drains:
        _split_drain_waits(nc)
    return nc


def prep_in_maps(x, edge, Wq, bq, Wk, bk, Wv, bv):
    bf16 = ml_dtypes.bfloat16
    x = np.ascontiguousarray(np.asarray(x, np.float32))
    edge = np.asarray(edge)
    xTr = np.ascontiguousarray(x.T.reshape(2, 128, N).transpose(1, 0, 2))

    def wprep(W):
        return np.ascontiguousarray(
            np.asarray(W, np.float32).reshape(2, 128, D).transpose(1, 0, 2)
        )

    def bprep(b):
        return np.ascontiguousarray(np.asarray(b, np.float32).reshape(2, 128).T)

    common = {
        "xTr": xTr,
        "wq": wprep(Wq),
        "wk": wprep(Wk),
        "wv": wprep(Wv),
        "bqc": bprep(bq),
        "bkc": bprep(bk),
        "bvb": np.ascontiguousarray(
            np.broadcast_to(np.asarray(bv, np.float32), (128, D))
        ),
        "ident": np.eye(128, dtype=np.float32),
        "identb": np.eye(128, dtype=np.float32).astype(bf16),
    }
    if VARIANT == "pemask":
        # additive mask: 0 where edge==1, -80000 (pre-exp-scale) where edge==0
        edge_bf = ((edge.astype(np.float32) - 1.0) * 80000.0).astype(bf16)
    else:
        edge_bf = edge.astype(bf16)
    in_maps = []
    for core in range(NCORES):
        n0 = core * NLOC
        m = dict(common)
        m["xq"] = np.ascontiguousarray(xTr[:, :, n0 : n0 + NLOC])
        m["edge"] = np.ascontiguousarray(edge_bf[:, n0 : n0 + NLOC])
        in_maps.append(m)
    return in_maps


_CACHED_NC = None


def kernel(x, edge, Wq, bq, Wk, bk, Wv, bv):
    global _CACHED_NC
    if _CACHED_NC is None:
        _CACHED_NC = build_module()
    nc = _CACHED_NC
    in_maps = prep_in_maps(x, edge, Wq, bq, Wk, bk, Wv, bv)
    res = bass_utils.run_bass_kernel_spmd(nc, in_maps, core_ids=list(range(NCORES)))
    out = np.concatenate([r["out"] for r in res.results], axis=0)
    return out.astype(np.float32)


if __name__ == "__main__":
    rng = np.random.default_rng(0)
    x = rng.standard_normal((N, D), dtype=np.float32)
    edge = rng.integers(0, 2, size=(N, N)).astype(np.int32)
    mk = lambda *s: (rng.standard_normal(s, dtype=np.float32) / 16.0)
    o = kernel(
        x, edge, mk(D, D), mk(D) * 0.16, mk(D, D), mk(D) * 0.16, mk(D, D), mk(D) * 0.16
    )
    print(o.shape, o.dtype)



# revision 12
# speedup vs baseline: 1.5761x; 1.1003x over previous
"""GAT layer (4-head masked attention over an 8192-node graph) on 8 TRN2 NeuronCores.

Sharding: query/node dim N=8192 split across 8 cores (1024 rows each); K/V and
projection weights replicated. Per core the kernel computes, in transposed-score
layout ST[m, n] (partition = key index m, free = query index n):

    QT/KT = W.T @ x.T        (fp32r matmuls, 2 heads packed per 128 partitions)
    V' = x @ Wv + bv         (bf16, with a ones-column per head appended)
    ST = K_h Q_h^T / 8       (fp32r, two 64-contraction row-tiled matmuls)
    P  = exp(ST) * edge      (ACT exp -> bf16, DVE masked multiply; no row-max
                              subtraction needed: scores are O(1) by construction)
    ctxT = V'^T @ P          (bf16; the ones-column makes row 64 the softmax
                              denominator, so no separate reduction is needed)
    out = (ctxT rows 0:64) / (ctxT row 64), PE-transposed back to [n, d].

Host side only reshapes/slices inputs, converts the {0,1} edge mask to bf16,
and concatenates per-core outputs.
"""

import sys

if "/opt/trn_rl_repo" not in sys.path:
    sys.path.insert(0, "/opt/trn_rl_repo")

import numpy as np
import ml_dtypes

import concourse.bass as bass
import concourse.tile as tile
from concourse import mybir
from concourse import bass_utils

N = 8192
D = 256
H = 4
DH = 64
NCORES = 8
NLOC = N // NCORES          # 1024 query rows per core
MT = N // 128               # 64 key tiles
VROW = H * (DH + 1)         # 260: per key-tile V' row: 4 heads x (64 V cols + ones)

F32 = mybir.dt.float32
F32R = mybir.dt.float32r
BF16 = mybir.dt.bfloat16

import os as _os
MAX_DRAIN_WAITS = 1  # HW-tested: 2 waits/inst fails codegen on every encoding here
# headout (kept for reference, NOT default): head-outer order frees PSUM for
# 1536-wide score tiles (176 exp/mask instructions instead of 256/512).
# HW-tested 2026-08-09: LOSES ~40% (434us vs 296us interleaved A/B) — ACT
# throughput from PSUM degrades with instruction width (~1.0 cyc/elem at
# 1024-wide, ~1.36 at 2048, ~1.7 at 1536 measured three ways), so the
# overhead saving is swamped. 1024-wide exp is the HW sweet spot; PV
# software-pipelining and flat-2D APs (both tried) don't close the gap.
VARIANT = _os.environ.get("KERNEL_VARIANT", "base")


def _split_drain_waits(nc):
    """walrus in this container rejects >1 sync-wait on several instruction
    encodings (Drain/TPB_CTRL_NO_STRUCT, fp32 matmul/S3_LW_STRUCT, ...).
    Hoist excess waits onto preceding same-engine InstNoOp carriers — the
    engine executes them in order, so semantics are unchanged."""
    for fn in nc.m.functions:
        for bb in fn.blocks:
            new_insts = []
            for inst in bb.instructions:
                si = inst.sync_info
                waits = list(si.on_wait) if si and si.on_wait else []
                if len(waits) > MAX_DRAIN_WAITS:
                    groups = [
                        waits[i : i + MAX_DRAIN_WAITS]
                        for i in range(0, len(waits), MAX_DRAIN_WAITS)
                    ]
                    for g in groups[:-1]:
                        d = mybir.InstNoOp(
                            name=nc.get_next_instruction_name(),
                            ins=[],
                            outs=[],
                        )
                        d.engine = inst.engine
                        d.sync_info = mybir.SyncInfo(on_wait=g, on_update=[])
                        new_insts.append(d)
                    inst.sync_info = mybir.SyncInfo(
                        on_wait=groups[-1], on_update=list(si.on_update)
                    )
                new_insts.append(inst)
            bb.instructions = new_insts


def build_module(split_drains=True, attn_reps=1, attn_loop=None, variant=None):
    var = VARIANT if variant is None else variant
    nc = bass.Bass(
        "TRN2",
        target_bir_lowering=False,
        debug=False,
        enable_asserts=True,
        num_devices=NCORES,
    )

    # Per-core DRAM I/O. xTr[p, j, m] = x[m, 128j+p]; w*[p, j, o] = W[128j+p, o].
    xTr = nc.dram_tensor("xTr", [128, 2, N], F32R, kind="ExternalInput").ap()
    xq = nc.dram_tensor("xq", [128, 2, NLOC], F32R, kind="ExternalInput").ap()
    edge = nc.dram_tensor("edge", [N, NLOC], BF16, kind="ExternalInput").ap()
    wq = nc.dram_tensor("wq", [128, 2, D], F32R, kind="ExternalInput").ap()
    wk = nc.dram_tensor("wk", [128, 2, D], F32R, kind="ExternalInput").ap()
    wv = nc.dram_tensor("wv", [128, 2, D], F32R, kind="ExternalInput").ap()
    bqc = nc.dram_tensor("bqc", [128, 2], F32, kind="ExternalInput").ap()
    bkc = nc.dram_tensor("bkc", [128, 2], F32, kind="ExternalInput").ap()
    bvb = nc.dram_tensor("bvb", [128, D], F32, kind="ExternalInput").ap()
    ident = nc.dram_tensor("ident", [128, 128], F32, kind="ExternalInput").ap()
    identb = nc.dram_tensor("identb", [128, 128], BF16, kind="ExternalInput").ap()
    out = nc.dram_tensor("out", [NLOC, D], F32, kind="ExternalOutput").ap()

    Ident = mybir.ActivationFunctionType.Identity
    Exp = mybir.ActivationFunctionType.Exp

    ho = var == "headout"
    with tile.TileContext(nc) as tc:
        with (
            tc.tile_pool(name="const", bufs=1) as cpool,
            tc.tile_pool(name="big", bufs=1) as bigpool,
            tc.tile_pool(name="xs", bufs=3) as xpool,
            tc.tile_pool(
                name="edge", bufs=(22 if ho else 6 if var == "bufs6" else 4)
            ) as epool,
            tc.tile_pool(name="p", bufs=(6 if ho else 8)) as ppool,
            tc.tile_pool(name="ctx", bufs=2) as ctxpool,
            tc.tile_pool(name="outs", bufs=3) as opool,
            tc.tile_pool(name="rc", bufs=8) as rpool,
            tc.tile_pool(
                name="ps_s", bufs=(1 if var == "batch2" else 2), space="PSUM"
            ) as ps_s,
            tc.tile_pool(name="ps_pv", bufs=(2 if ho else 4), space="PSUM") as ps_pv,
        ):
            # ---- constants ----
            wq_sb = cpool.tile([128, 2, D], F32R, tag="wq")
            nc.sync.dma_start(wq_sb[:], wq[:])
            wk_sb = cpool.tile([128, 2, D], F32R, tag="wk")
            nc.sync.dma_start(wk_sb[:], wk[:])
            wv_sb = cpool.tile([128, 2, D], F32R, tag="wv")
            nc.sync.dma_start(wv_sb[:], wv[:])
            bq_sb = cpool.tile([128, 2], F32, tag="bq")
            nc.sync.dma_start(bq_sb[:], bqc[:])
            bk_sb = cpool.tile([128, 2], F32, tag="bk")
            nc.sync.dma_start(bk_sb[:], bkc[:])
            bv_sb = cpool.tile([128, D], F32, tag="bv")
            nc.sync.dma_start(bv_sb[:], bvb[:])
            id_sb = cpool.tile([128, 128], F32, tag="id")
            nc.sync.dma_start(id_sb[:], ident[:])
            idb_sb = cpool.tile([128, 128], BF16, tag="idb")
            nc.sync.dma_start(idb_sb[:], identb[:])
            xq_sb = cpool.tile([128, 2, NLOC], F32R, tag="xq")
            nc.sync.dma_start(xq_sb[:], xq[:])

            QT = bigpool.tile([128, 2, NLOC], BF16, tag="qt")
            KT = bigpool.tile([128, 2, N], BF16, tag="kt")
            Vp = bigpool.tile([128, MT * VROW], BF16, tag="vp")
            nc.vector.memset(Vp[:], 1.0)

            # ---- Q projection: QT[p, j, n] = sum_d Wq[d, 128j+p] x[n0+n, d] + bq ----
            for j in range(2):
                if ho:
                    q_ps = ps_s.tile([128, 3, 512], F32, tag="sc")
                    qf = q_ps[:, :, :].rearrange("p a b -> p (a b)")
                else:
                    q_ps = ps_s.tile([128, NLOC], F32, tag="sc")
                    qf = q_ps[:]
                for c in range(2):
                    for ji in range(2):
                        nc.tensor.matmul(
                            qf[:, c * 512 : (c + 1) * 512],
                            lhsT=wq_sb[:, ji, j * 128 : (j + 1) * 128],
                            rhs=xq_sb[:, ji, c * 512 : (c + 1) * 512],
                            start=(ji == 0),
                            stop=(ji == 1),
                        )
                nc.scalar.activation(
                    QT[:, j, :], qf[:, 0:NLOC], Ident, bias=bq_sb[:, j : j + 1]
                )

            # ---- K/V projections, streaming x.T in 16 chunks of 512 keys ----
            for mc in range(16):
                xc = xpool.tile([128, 2, 512], F32R, tag="xc")
                nc.sync.dma_start(xc[:], xTr[:, :, mc * 512 : (mc + 1) * 512])
                for j in range(2):
                    k_ps = ps_pv.tile([128, 512], F32, tag="pv")
                    for ji in range(2):
                        nc.tensor.matmul(
                            k_ps[:],
                            lhsT=wk_sb[:, ji, j * 128 : (j + 1) * 128],
                            rhs=xc[:, ji, :],
                            start=(ji == 0),
                            stop=(ji == 1),
                        )
                    nc.scalar.activation(
                        KT[:, j, mc * 512 : (mc + 1) * 512],
                        k_ps[:],
                        Ident,
                        bias=bk_sb[:, j : j + 1],
                    )
                for mt in range(4):
                    t = mc * 4 + mt
                    v_ps = ps_pv.tile([128, D], F32, tag="pv")
                    for ji in range(2):
                        nc.tensor.matmul(
                            v_ps[:],
                            lhsT=xc[:, ji, mt * 128 : (mt + 1) * 128],
                            rhs=wv_sb[:, ji, :],
                            start=(ji == 0),
                            stop=(ji == 1),
                        )
                    out_v = Vp[:, t * VROW : (t + 1) * VROW].rearrange(
                        "p (h q) -> p h q", h=H
                    )[:, :, 0:DH]
                    nc.vector.tensor_add(
                        out_v,
                        v_ps[:].rearrange("p (h q) -> p h q", h=H),
                        bv_sb[:].rearrange("p (h q) -> p h q", h=H),
                    )

            def emit_attention():
                # ---- attention over the core's NLOC queries, in two 512 chunks ----
                # attn_reps>1 repeats the (idempotent) attention phase for timing.
                for rep in range(attn_reps):
                  for c in range(2):
                    n0 = c * 512
                    pv_ps = [
                        ps_pv.tile([128, 512], F32, tag="pv", name=f"pv_{rep}_{c}_{h}")
                        for h in range(H)
                    ]
                    if VARIANT == "batch2":
                        # 2 m-tiles per step: one [128,2048] score tile (4 PSUM
                        # banks, single-buffered) and one exp per (pair, 2 tiles)
                        # halves the ACT per-instruction overhead.
                        for tb in range(MT // 2):
                            e2 = epool.tile([128, 2, 512], BF16, tag="e")
                            nc.sync.dma_start(
                                e2[:],
                                edge[
                                    tb * 256 : (tb + 1) * 256, n0 : n0 + 512
                                ].rearrange("(tt p) q -> p tt q", p=128),
                            )
                            for pair in range(2):
                                s_ps = ps_s.tile([128, 2048], F32, tag="sc")
                                for i in range(2):
                                    po = i * 64
                                    for tt in range(2):
                                        t = 2 * tb + tt
                                        nc.tensor.matmul(
                                            s_ps[
                                                :,
                                                i * 1024
                                                + tt * 512 : i * 1024
                                                + (tt + 1) * 512,
                                            ],
                                            lhsT=KT[
                                                po : po + 64,
                                                pair,
                                                t * 128 : (t + 1) * 128,
                                            ],
                                            rhs=QT[
                                                po : po + 64, pair, n0 : n0 + 512
                                            ],
                                            start=True,
                                            stop=True,
                                        )
                                p_sb = ppool.tile([128, 2048], BF16, tag="p")
                                nc.scalar.activation(p_sb[:], s_ps[:], Exp, scale=0.125)
                                for i in range(2):
                                    for tt in range(2):
                                        nc.vector.tensor_mul(
                                            p_sb[
                                                :,
                                                i * 1024
                                                + tt * 512 : i * 1024
                                                + (tt + 1) * 512,
                                            ],
                                            p_sb[
                                                :,
                                                i * 1024
                                                + tt * 512 : i * 1024
                                                + (tt + 1) * 512,
                                            ],
                                            e2[:, tt, :],
                                        )
                                for i in range(2):
                                    h = 2 * pair + i
                                    for tt in range(2):
                                        t = 2 * tb + tt
                                        nc.tensor.matmul(
                                            pv_ps[h][0:65, :],
                                            lhsT=Vp[
                                                :,
                                                t * VROW
                                                + h * 65 : t * VROW
                                                + h * 65
                                                + 65,
                                            ],
                                            rhs=p_sb[
                                                :,
                                                i * 1024
                                                + tt * 512 : i * 1024
                                                + (tt + 1) * 512,
                                            ],
                                            start=(t == 0),
                                            stop=(t == MT - 1),
                                        )
                    for t in range(MT if var != "batch2" else 0):
                        if var == "edup":
                            # edge tile duplicated side-by-side so the mask
                            # multiply is ONE flat [128,1024] DVE op (no
                            # stride-0 broadcast, which costs +19us/pass)
                            e_sb = epool.tile([128, 2, 512], BF16, tag="e")
                            for du in range(2):
                                nc.sync.dma_start(
                                    e_sb[:, du, :],
                                    edge[t * 128 : (t + 1) * 128, n0 : n0 + 512],
                                )
                        else:
                            e_sb = epool.tile([128, 512], BF16, tag="e")
                            if var != "nodma":
                                nc.sync.dma_start(
                                    e_sb[:],
                                    edge[t * 128 : (t + 1) * 128, n0 : n0 + 512],
                                )
                        for pair in range(2):
                            s_ps = ps_s.tile([128, 1024], F32, tag="sc")
                            pe_mask = VARIANT == "pemask"
                            for i in range(2 if VARIANT != "noqk" else 0):
                                po = i * 64
                                nc.tensor.matmul(
                                    s_ps[:, i * 512 : (i + 1) * 512],
                                    lhsT=KT[
                                        po : po + 64, pair, t * 128 : (t + 1) * 128
                                    ],
                                    rhs=QT[po : po + 64, pair, n0 : n0 + 512],
                                    start=True,
                                    stop=not pe_mask,
                                )
                            if pe_mask:
                                # accumulate the additive -inf mask via the PE:
                                # s_ps[m, n] += sum_k I[k, m] * M[k, n] = M[m, n]
                                for i in range(2):
                                    nc.tensor.matmul(
                                        s_ps[:, i * 512 : (i + 1) * 512],
                                        lhsT=idb_sb[:],
                                        rhs=e_sb[:],
                                        start=False,
                                        stop=True,
                                    )
                            p_sb = ppool.tile([128, 1024], BF16, tag="p")
                            if VARIANT == "noexp":
                                nc.vector.tensor_copy(p_sb[:], s_ps[:])
                            else:
                                nc.scalar.activation(
                                    p_sb[:], s_ps[:], Exp, scale=0.125
                                )
                            if var in ("nomask", "pemask"):
                                pass
                            elif var == "edup":
                                nc.vector.tensor_mul(
                                    p_sb[:],
                                    p_sb[:],
                                    e_sb[:, :, :].rearrange("p a b -> p (a b)"),
                                )
                            else:
                                # two plain [128, 512] muls: a broadcast in1 AP
                                # degrades the DVE perf-mode (HW-measured +19us
                                # per pass vs this form)
                                for i in range(2):
                                    nc.vector.tensor_mul(
                                        p_sb[:, i * 512 : (i + 1) * 512],
                                        p_sb[:, i * 512 : (i + 1) * 512],
                                        e_sb[:],
                                    )
                            for i in range(2 if VARIANT != "nopv" else 0):
                                h = 2 * pair + i
                                nc.tensor.matmul(
                                    pv_ps[h][0:65, :],
                                    lhsT=Vp[
                                        :, t * VROW + h * 65 : t * VROW + h * 65 + 65
                                    ],
                                    rhs=p_sb[:, i * 512 : (i + 1) * 512],
                                    start=(t == 0),
                                    stop=(t == MT - 1),
                                )

                    # epilogue: divide by denominator row, transpose to [n, d]
                    ctx = ctxpool.tile([128, H * 512], F32, tag="ctx")
                    for h in range(H):
                        nc.vector.tensor_copy(
                            ctx[0:65, h * 512 : (h + 1) * 512], pv_ps[h][0:65, :]
                        )
                    for sub in range(4):
                        tr_ps = ps_s.tile([128, 1024], F32, tag="sc")
                        for h in range(H):
                            nc.tensor.transpose(
                                tr_ps[:, h * 65 : h * 65 + 65],
                                ctx[0:65, h * 512 + sub * 128 : h * 512 + (sub + 1) * 128],
                                id_sb[0:65, 0:65],
                            )
                        o_sb = opool.tile([128, D], F32, tag="o")
                        for h in range(H):
                            rc = rpool.tile([128, 1], F32, tag="rc")
                            nc.vector.reciprocal(
                                rc[:], tr_ps[:, h * 65 + 64 : h * 65 + 65]
                            )
                            nc.vector.tensor_scalar_mul(
                                o_sb[:, h * DH : (h + 1) * DH],
                                tr_ps[:, h * 65 : h * 65 + 64],
                                rc[:],
                            )
                        nc.sync.dma_start(
                            out[n0 + sub * 128 : n0 + (sub + 1) * 128, :], o_sb[:]
                        )

            def emit_attention_headout():
                # head-outer: one head's full key sweep at a time. Only one PV
                # accumulator bank is live (+1 for overlap), so score tiles can
                # span 3 key-tiles (1536 fp32 = 3 PSUM banks, double-buffered):
                # 176 exp instructions/pass instead of 256, 176 mask-muls
                # instead of 512. Edge tiles are DMA'd once per chunk (h==0)
                # and re-read by the other 3 heads from SBUF.
                G = (MT + 2) // 3  # 22 tiles of up to 3 key-tiles each
                for rep in range(attn_reps):
                  for c in range(2):
                    n0 = c * 512
                    ctx = ctxpool.tile([128, H * 512], F32, tag="ctx")
                    # hoist the chunk's edge DMAs so h==0 mask-muls never wait
                    # on DMA issue order
                    etiles = []
                    for g in range(G):
                        t0 = 3 * g
                        tt = min(3, MT - t0)
                        e3 = epool.tile([128, 3, 512], BF16, tag="e")
                        nc.sync.dma_start(
                            e3[:, 0:tt, :],
                            edge[
                                t0 * 128 : (t0 + tt) * 128, n0 : n0 + 512
                            ].rearrange("(tt p) q -> p tt q", p=128),
                        )
                        etiles.append(e3)
                    for h in range(H):
                        pair, i = divmod(h, 2)
                        po = i * 64
                        pv = ps_pv.tile(
                            [128, 512], F32, tag="pv", name=f"pv_{rep}_{c}_{h}"
                        )

                        def emit_pv(t0, tt, p_sb):
                            for k in range(tt):
                                t = t0 + k
                                nc.tensor.matmul(
                                    pv[0:65, :],
                                    lhsT=Vp[
                                        :, t * VROW + h * 65 : t * VROW + h * 65 + 65
                                    ],
                                    rhs=p_sb[:, k, :],
                                    start=(t == 0),
                                    stop=(t == MT - 1),
                                )

                        # PV runs one tile behind QK/exp/mask: its mask is long
                        # done by then, so no PE instruction ever parks on a
                        # cross-engine wait (PE wait-queue depth is 4).
                        prev = None
                        for g in range(G):
                            t0 = 3 * g
                            tt = min(3, MT - t0)
                            e3 = etiles[g]
                            s_ps = ps_s.tile([128, 3, 512], F32, tag="sc")
                            for k in range(tt):
                                t = t0 + k
                                nc.tensor.matmul(
                                    s_ps[:, k, :],
                                    lhsT=KT[
                                        po : po + 64, pair, t * 128 : (t + 1) * 128
                                    ],
                                    rhs=QT[po : po + 64, pair, n0 : n0 + 512],
                                    start=True,
                                    stop=True,
                                )
                            p_sb = ppool.tile([128, 3, 512], BF16, tag="p")
                            # flatten to 2-D contiguous APs: engines only hit
                            # their fast path / documented rate on flat APs
                            pf = p_sb[:, 0:tt, :].rearrange("p a b -> p (a b)")
                            sf = s_ps[:, 0:tt, :].rearrange("p a b -> p (a b)")
                            ef = e3[:, 0:tt, :].rearrange("p a b -> p (a b)")
                            nc.scalar.activation(pf, sf, Exp, scale=0.125)
                            nc.vector.tensor_mul(pf, pf, ef)
                            if prev is not None:
                                emit_pv(*prev)
                            prev = (t0, tt, p_sb)
                        emit_pv(*prev)
                        nc.vector.tensor_copy(
                            ctx[0:65, h * 512 : (h + 1) * 512], pv[0:65, :]
                        )
                    # epilogue: divide by denominator row, transpose to [n, d]
                    for sub in range(4):
                        tr_ps = ps_s.tile([128, 3, 512], F32, tag="sc")
                        trf = tr_ps[:, :, :].rearrange("p a b -> p (a b)")
                        for h in range(H):
                            nc.tensor.transpose(
                                trf[:, h * 65 : h * 65 + 65],
                                ctx[
                                    0:65,
                                    h * 512 + sub * 128 : h * 512 + (sub + 1) * 128,
                                ],
                                id_sb[0:65, 0:65],
                            )
                        o_sb = opool.tile([128, D], F32, tag="o")
                        for h in range(H):
                            rc = rpool.tile([128, 1], F32, tag="rc")
                            nc.vector.reciprocal(
                                rc[:], trf[:, h * 65 + 64 : h * 65 + 65]
                            )
                            nc.vector.tensor_scalar_mul(
                                o_sb[:, h * DH : (h + 1) * DH],
                                trf[:, h * 65 : h * 65 + 64],
                                rc[:],
                            )
                        nc.sync.dma_start(
                            out[n0 + sub * 128 : n0 + (sub + 1) * 128, :], o_sb[:]
                        )

            emit = emit_attention_headout if ho else emit_attention
            if attn_loop is None:
                emit()
            else:
                with tc.For_i(0, attn_loop, 1):
                    emit()

    if split_drains:
        _split_drain_waits(nc)
    return nc


def prep_in_maps(x, edge, Wq, bq, Wk, bk, Wv, bv):
    bf16 = ml_dtypes.bfloat16
    x = np.ascontiguousarray(np.asarray(x, np.float32))
    edge = np.asarray(edge)
    xTr = np.ascontiguousarray(x.T.reshape(2, 128, N).transpose(1, 0, 2))

    def wprep(W):
        return np.ascontiguousarray(
            np.asarray(W, np.float32).reshape(2, 128, D).transpose(1, 0, 2)
        )

    def bprep(b):
        return np.ascontiguousarray(np.asarray(b, np.float32).reshape(2, 128).T)

    common = {
        "xTr": xTr,
        "wq": wprep(Wq),
        "wk": wprep(Wk),
        "wv": wprep(Wv),
        "bqc": bprep(bq),
        "bkc": bprep(bk),
        "bvb": np.ascontiguousarray(
            np.broadcast_to(np.asarray(bv, np.float32), (128, D))
        ),
        "ident": np.eye(128, dtype=np.float32),
        "identb": np.eye(128, dtype=np.float32).astype(bf16),
    }
    if VARIANT == "pemask":
        # additive mask: 0 where edge==1, -80000 (pre-exp-scale) where edge==0
        edge_bf = ((edge.astype(np.float32) - 1.0) * 80000.0).astype(bf16)
    else:
        edge_bf = edge.astype(bf16)
    in_maps = []
    for core in range(NCORES):
        n0 = core * NLOC
        m = dict(common)
        m["xq"] = np.ascontiguousarray(xTr[:, :, n0 : n0 + NLOC])
        m["edge"] = np.ascontiguousarray(edge_bf[:, n0 : n0 + NLOC])
        in_maps.append(m)
    return in_maps


_CACHED_NC = None


def kernel(x, edge, Wq, bq, Wk, bk, Wv, bv):
    global _CACHED_NC
    if _CACHED_NC is None:
        _CACHED_NC = build_module()
    nc = _CACHED_NC
    in_maps = prep_in_maps(x, edge, Wq, bq, Wk, bk, Wv, bv)
    res = bass_utils.run_bass_kernel_spmd(nc, in_maps, core_ids=list(range(NCORES)))
    out = np.concatenate([r["out"] for r in res.results], axis=0)
    return out.astype(np.float32)


if __name__ == "__main__":
    rng = np.random.default_rng(0)
    x = rng.standard_normal((N, D), dtype=np.float32)
    edge = rng.integers(0, 2, size=(N, N)).astype(np.int32)
    mk = lambda *s: (rng.standard_normal(s, dtype=np.float32) / 16.0)
    o = kernel(
        x, edge, mk(D, D), mk(D) * 0.16, mk(D, D), mk(D) * 0.16, mk(D, D), mk(D) * 0.16
    )
    print(o.shape, o.dtype)



# revision 13
# speedup vs baseline: 1.6925x; 1.0738x over previous
"""GAT layer (4-head masked attention over an 8192-node graph) on 8 TRN2 NeuronCores.

Sharding: query/node dim N=8192 split across 8 cores (1024 rows each); K/V and
projection weights replicated. Per core the kernel computes, in transposed-score
layout ST[m, n] (partition = key index m, free = query index n):

    QT/KT = W.T @ x.T        (fp32r matmuls, 2 heads packed per 128 partitions)
    V' = x @ Wv + bv         (bf16, with a ones-column per head appended)
    ST = K_h Q_h^T / 8       (fp32r, two 64-contraction row-tiled matmuls)
    P  = exp(ST) * edge      (ACT exp -> bf16, DVE masked multiply; no row-max
                              subtraction needed: scores are O(1) by construction)
    ctxT = V'^T @ P          (bf16; the ones-column makes row 64 the softmax
                              denominator, so no separate reduction is needed)
    out = (ctxT rows 0:64) / (ctxT row 64), PE-transposed back to [n, d].

Host side only reshapes/slices inputs, converts the {0,1} edge mask to bf16,
and concatenates per-core outputs.
"""

import sys

if "/opt/trn_rl_repo" not in sys.path:
    sys.path.insert(0, "/opt/trn_rl_repo")

import numpy as np
import ml_dtypes

import concourse.bass as bass
import concourse.tile as tile
from concourse import mybir
from concourse import bass_utils

N = 8192
D = 256
H = 4
DH = 64
NCORES = 8
NLOC = N // NCORES          # 1024 query rows per core
MT = N // 128               # 64 key tiles
VROW = H * (DH + 1)         # 260: per key-tile V' row: 4 heads x (64 V cols + ones)

F32 = mybir.dt.float32
F32R = mybir.dt.float32r
BF16 = mybir.dt.bfloat16

import os as _os
MAX_DRAIN_WAITS = 1  # HW-tested: 2 waits/inst fails codegen on every encoding here
# headout (kept for reference, NOT default): head-outer order frees PSUM for
# 1536-wide score tiles (176 exp/mask instructions instead of 256/512).
# HW-tested 2026-08-09: LOSES ~40% (434us vs 296us interleaved A/B) — ACT
# throughput from PSUM degrades with instruction width (~1.0 cyc/elem at
# 1024-wide, ~1.36 at 2048, ~1.7 at 1536 measured three ways), so the
# overhead saving is swamped. 1024-wide exp is the HW sweet spot; PV
# software-pipelining and flat-2D APs (both tried) don't close the gap.
# default bufs6: identical instruction stream to base, edge-DMA ring 6 deep
# instead of 4. Interleaved A/B under load: 288.0/274.0 us (med/min-diff) vs
# base 293.5/293.7 — prefetch slack when DMA is contended, ties base when quiet.
VARIANT = _os.environ.get("KERNEL_VARIANT", "bufs6")


def _split_drain_waits(nc):
    """walrus in this container rejects >1 sync-wait on several instruction
    encodings (Drain/TPB_CTRL_NO_STRUCT, fp32 matmul/S3_LW_STRUCT, ...).
    Hoist excess waits onto preceding same-engine InstNoOp carriers — the
    engine executes them in order, so semantics are unchanged."""
    for fn in nc.m.functions:
        for bb in fn.blocks:
            new_insts = []
            for inst in bb.instructions:
                si = inst.sync_info
                waits = list(si.on_wait) if si and si.on_wait else []
                if len(waits) > MAX_DRAIN_WAITS:
                    groups = [
                        waits[i : i + MAX_DRAIN_WAITS]
                        for i in range(0, len(waits), MAX_DRAIN_WAITS)
                    ]
                    for g in groups[:-1]:
                        d = mybir.InstNoOp(
                            name=nc.get_next_instruction_name(),
                            ins=[],
                            outs=[],
                        )
                        d.engine = inst.engine
                        d.sync_info = mybir.SyncInfo(on_wait=g, on_update=[])
                        new_insts.append(d)
                    inst.sync_info = mybir.SyncInfo(
                        on_wait=groups[-1], on_update=list(si.on_update)
                    )
                new_insts.append(inst)
            bb.instructions = new_insts


def build_module(split_drains=True, attn_reps=1, attn_loop=None, variant=None):
    var = VARIANT if variant is None else variant
    nc = bass.Bass(
        "TRN2",
        target_bir_lowering=False,
        debug=False,
        enable_asserts=True,
        num_devices=NCORES,
    )

    # Per-core DRAM I/O. xTr[p, j, m] = x[m, 128j+p]; w*[p, j, o] = W[128j+p, o].
    xTr = nc.dram_tensor("xTr", [128, 2, N], F32R, kind="ExternalInput").ap()
    xq = nc.dram_tensor("xq", [128, 2, NLOC], F32R, kind="ExternalInput").ap()
    edge = nc.dram_tensor("edge", [N, NLOC], BF16, kind="ExternalInput").ap()
    wq = nc.dram_tensor("wq", [128, 2, D], F32R, kind="ExternalInput").ap()
    wk = nc.dram_tensor("wk", [128, 2, D], F32R, kind="ExternalInput").ap()
    wv = nc.dram_tensor("wv", [128, 2, D], F32R, kind="ExternalInput").ap()
    bqc = nc.dram_tensor("bqc", [128, 2], F32, kind="ExternalInput").ap()
    bkc = nc.dram_tensor("bkc", [128, 2], F32, kind="ExternalInput").ap()
    bvb = nc.dram_tensor("bvb", [128, D], F32, kind="ExternalInput").ap()
    ident = nc.dram_tensor("ident", [128, 128], F32, kind="ExternalInput").ap()
    identb = nc.dram_tensor("identb", [128, 128], BF16, kind="ExternalInput").ap()
    out = nc.dram_tensor("out", [NLOC, D], F32, kind="ExternalOutput").ap()

    Ident = mybir.ActivationFunctionType.Identity
    Exp = mybir.ActivationFunctionType.Exp

    ho = var == "headout"
    with tile.TileContext(nc) as tc:
        with (
            tc.tile_pool(name="const", bufs=1) as cpool,
            tc.tile_pool(name="big", bufs=1) as bigpool,
            tc.tile_pool(name="xs", bufs=3) as xpool,
            tc.tile_pool(
                name="edge", bufs=(22 if ho else 6 if var == "bufs6" else 4)
            ) as epool,
            tc.tile_pool(name="p", bufs=(6 if ho else 8)) as ppool,
            tc.tile_pool(name="ctx", bufs=2) as ctxpool,
            tc.tile_pool(name="outs", bufs=3) as opool,
            tc.tile_pool(name="rc", bufs=8) as rpool,
            tc.tile_pool(
                name="ps_s", bufs=(1 if var == "batch2" else 2), space="PSUM"
            ) as ps_s,
            tc.tile_pool(name="ps_pv", bufs=(2 if ho else 4), space="PSUM") as ps_pv,
        ):
            # ---- constants ----
            wq_sb = cpool.tile([128, 2, D], F32R, tag="wq")
            nc.sync.dma_start(wq_sb[:], wq[:])
            wk_sb = cpool.tile([128, 2, D], F32R, tag="wk")
            nc.sync.dma_start(wk_sb[:], wk[:])
            wv_sb = cpool.tile([128, 2, D], F32R, tag="wv")
            nc.sync.dma_start(wv_sb[:], wv[:])
            bq_sb = cpool.tile([128, 2], F32, tag="bq")
            nc.sync.dma_start(bq_sb[:], bqc[:])
            bk_sb = cpool.tile([128, 2], F32, tag="bk")
            nc.sync.dma_start(bk_sb[:], bkc[:])
            bv_sb = cpool.tile([128, D], F32, tag="bv")
            nc.sync.dma_start(bv_sb[:], bvb[:])
            id_sb = cpool.tile([128, 128], F32, tag="id")
            nc.sync.dma_start(id_sb[:], ident[:])
            idb_sb = cpool.tile([128, 128], BF16, tag="idb")
            nc.sync.dma_start(idb_sb[:], identb[:])
            xq_sb = cpool.tile([128, 2, NLOC], F32R, tag="xq")
            nc.sync.dma_start(xq_sb[:], xq[:])

            QT = bigpool.tile([128, 2, NLOC], BF16, tag="qt")
            KT = bigpool.tile([128, 2, N], BF16, tag="kt")
            Vp = bigpool.tile([128, MT * VROW], BF16, tag="vp")
            nc.vector.memset(Vp[:], 1.0)

            # ---- Q projection: QT[p, j, n] = sum_d Wq[d, 128j+p] x[n0+n, d] + bq ----
            for j in range(2):
                if ho:
                    q_ps = ps_s.tile([128, 3, 512], F32, tag="sc")
                    qf = q_ps[:, :, :].rearrange("p a b -> p (a b)")
                else:
                    q_ps = ps_s.tile([128, NLOC], F32, tag="sc")
                    qf = q_ps[:]
                for c in range(2):
                    for ji in range(2):
                        nc.tensor.matmul(
                            qf[:, c * 512 : (c + 1) * 512],
                            lhsT=wq_sb[:, ji, j * 128 : (j + 1) * 128],
                            rhs=xq_sb[:, ji, c * 512 : (c + 1) * 512],
                            start=(ji == 0),
                            stop=(ji == 1),
                        )
                nc.scalar.activation(
                    QT[:, j, :], qf[:, 0:NLOC], Ident, bias=bq_sb[:, j : j + 1]
                )

            # ---- K/V projections, streaming x.T in 16 chunks of 512 keys ----
            for mc in range(16):
                xc = xpool.tile([128, 2, 512], F32R, tag="xc")
                nc.sync.dma_start(xc[:], xTr[:, :, mc * 512 : (mc + 1) * 512])
                for j in range(2):
                    k_ps = ps_pv.tile([128, 512], F32, tag="pv")
                    for ji in range(2):
                        nc.tensor.matmul(
                            k_ps[:],
                            lhsT=wk_sb[:, ji, j * 128 : (j + 1) * 128],
                            rhs=xc[:, ji, :],
                            start=(ji == 0),
                            stop=(ji == 1),
                        )
                    nc.scalar.activation(
                        KT[:, j, mc * 512 : (mc + 1) * 512],
                        k_ps[:],
                        Ident,
                        bias=bk_sb[:, j : j + 1],
                    )
                for mt in range(4):
                    t = mc * 4 + mt
                    v_ps = ps_pv.tile([128, D], F32, tag="pv")
                    for ji in range(2):
                        nc.tensor.matmul(
                            v_ps[:],
                            lhsT=xc[:, ji, mt * 128 : (mt + 1) * 128],
                            rhs=wv_sb[:, ji, :],
                            start=(ji == 0),
                            stop=(ji == 1),
                        )
                    out_v = Vp[:, t * VROW : (t + 1) * VROW].rearrange(
                        "p (h q) -> p h q", h=H
                    )[:, :, 0:DH]
                    nc.vector.tensor_add(
                        out_v,
                        v_ps[:].rearrange("p (h q) -> p h q", h=H),
                        bv_sb[:].rearrange("p (h q) -> p h q", h=H),
                    )

            def emit_attention():
                # ---- attention over the core's NLOC queries, in two 512 chunks ----
                # attn_reps>1 repeats the (idempotent) attention phase for timing.
                for rep in range(attn_reps):
                  for c in range(2):
                    n0 = c * 512
                    pv_ps = [
                        ps_pv.tile([128, 512], F32, tag="pv", name=f"pv_{rep}_{c}_{h}")
                        for h in range(H)
                    ]
                    if VARIANT == "batch2":
                        # 2 m-tiles per step: one [128,2048] score tile (4 PSUM
                        # banks, single-buffered) and one exp per (pair, 2 tiles)
                        # halves the ACT per-instruction overhead.
                        for tb in range(MT // 2):
                            e2 = epool.tile([128, 2, 512], BF16, tag="e")
                            nc.sync.dma_start(
                                e2[:],
                                edge[
                                    tb * 256 : (tb + 1) * 256, n0 : n0 + 512
                                ].rearrange("(tt p) q -> p tt q", p=128),
                            )
                            for pair in range(2):
                                s_ps = ps_s.tile([128, 2048], F32, tag="sc")
                                for i in range(2):
                                    po = i * 64
                                    for tt in range(2):
                                        t = 2 * tb + tt
                                        nc.tensor.matmul(
                                            s_ps[
                                                :,
                                                i * 1024
                                                + tt * 512 : i * 1024
                                                + (tt + 1) * 512,
                                            ],
                                            lhsT=KT[
                                                po : po + 64,
                                                pair,
                                                t * 128 : (t + 1) * 128,
                                            ],
                                            rhs=QT[
                                                po : po + 64, pair, n0 : n0 + 512
                                            ],
                                            start=True,
                                            stop=True,
                                        )
                                p_sb = ppool.tile([128, 2048], BF16, tag="p")
                                nc.scalar.activation(p_sb[:], s_ps[:], Exp, scale=0.125)
                                for i in range(2):
                                    for tt in range(2):
                                        nc.vector.tensor_mul(
                                            p_sb[
                                                :,
                                                i * 1024
                                                + tt * 512 : i * 1024
                                                + (tt + 1) * 512,
                                            ],
                                            p_sb[
                                                :,
                                                i * 1024
                                                + tt * 512 : i * 1024
                                                + (tt + 1) * 512,
                                            ],
                                            e2[:, tt, :],
                                        )
                                for i in range(2):
                                    h = 2 * pair + i
                                    for tt in range(2):
                                        t = 2 * tb + tt
                                        nc.tensor.matmul(
                                            pv_ps[h][0:65, :],
                                            lhsT=Vp[
                                                :,
                                                t * VROW
                                                + h * 65 : t * VROW
                                                + h * 65
                                                + 65,
                                            ],
                                            rhs=p_sb[
                                                :,
                                                i * 1024
                                                + tt * 512 : i * 1024
                                                + (tt + 1) * 512,
                                            ],
                                            start=(t == 0),
                                            stop=(t == MT - 1),
                                        )
                    for t in range(MT if var != "batch2" else 0):
                        if var == "edup":
                            # edge tile duplicated side-by-side so the mask
                            # multiply is ONE flat [128,1024] DVE op (no
                            # stride-0 broadcast, which costs +19us/pass)
                            e_sb = epool.tile([128, 2, 512], BF16, tag="e")
                            for du in range(2):
                                nc.sync.dma_start(
                                    e_sb[:, du, :],
                                    edge[t * 128 : (t + 1) * 128, n0 : n0 + 512],
                                )
                        else:
                            e_sb = epool.tile([128, 512], BF16, tag="e")
                            if var != "nodma":
                                nc.sync.dma_start(
                                    e_sb[:],
                                    edge[t * 128 : (t + 1) * 128, n0 : n0 + 512],
                                )
                        for pair in range(2):
                            s_ps = ps_s.tile([128, 1024], F32, tag="sc")
                            pe_mask = VARIANT == "pemask"
                            for i in range(2 if VARIANT != "noqk" else 0):
                                po = i * 64
                                nc.tensor.matmul(
                                    s_ps[:, i * 512 : (i + 1) * 512],
                                    lhsT=KT[
                                        po : po + 64, pair, t * 128 : (t + 1) * 128
                                    ],
                                    rhs=QT[po : po + 64, pair, n0 : n0 + 512],
                                    start=True,
                                    stop=not pe_mask,
                                )
                            if pe_mask:
                                # accumulate the additive -inf mask via the PE:
                                # s_ps[m, n] += sum_k I[k, m] * M[k, n] = M[m, n]
                                for i in range(2):
                                    nc.tensor.matmul(
                                        s_ps[:, i * 512 : (i + 1) * 512],
                                        lhsT=idb_sb[:],
                                        rhs=e_sb[:],
                                        start=False,
                                        stop=True,
                                    )
                            p_sb = ppool.tile([128, 1024], BF16, tag="p")
                            if VARIANT == "noexp":
                                nc.vector.tensor_copy(p_sb[:], s_ps[:])
                            else:
                                nc.scalar.activation(
                                    p_sb[:], s_ps[:], Exp, scale=0.125
                                )
                            if var in ("nomask", "pemask"):
                                pass
                            elif var == "edup":
                                nc.vector.tensor_mul(
                                    p_sb[:],
                                    p_sb[:],
                                    e_sb[:, :, :].rearrange("p a b -> p (a b)"),
                                )
                            else:
                                # two plain [128, 512] muls: a broadcast in1 AP
                                # degrades the DVE perf-mode (HW-measured +19us
                                # per pass vs this form)
                                for i in range(2):
                                    nc.vector.tensor_mul(
                                        p_sb[:, i * 512 : (i + 1) * 512],
                                        p_sb[:, i * 512 : (i + 1) * 512],
                                        e_sb[:],
                                    )
                            for i in range(2 if VARIANT != "nopv" else 0):
                                h = 2 * pair + i
                                nc.tensor.matmul(
                                    pv_ps[h][0:65, :],
                                    lhsT=Vp[
                                        :, t * VROW + h * 65 : t * VROW + h * 65 + 65
                                    ],
                                    rhs=p_sb[:, i * 512 : (i + 1) * 512],
                                    start=(t == 0),
                                    stop=(t == MT - 1),
                                )

                    # epilogue: divide by denominator row, transpose to [n, d]
                    ctx = ctxpool.tile([128, H * 512], F32, tag="ctx")
                    for h in range(H):
                        nc.vector.tensor_copy(
                            ctx[0:65, h * 512 : (h + 1) * 512], pv_ps[h][0:65, :]
                        )
                    for sub in range(4):
                        tr_ps = ps_s.tile([128, 1024], F32, tag="sc")
                        for h in range(H):
                            nc.tensor.transpose(
                                tr_ps[:, h * 65 : h * 65 + 65],
                                ctx[0:65, h * 512 + sub * 128 : h * 512 + (sub + 1) * 128],
                                id_sb[0:65, 0:65],
                            )
                        o_sb = opool.tile([128, D], F32, tag="o")
                        for h in range(H):
                            rc = rpool.tile([128, 1], F32, tag="rc")
                            nc.vector.reciprocal(
                                rc[:], tr_ps[:, h * 65 + 64 : h * 65 + 65]
                            )
                            nc.vector.tensor_scalar_mul(
                                o_sb[:, h * DH : (h + 1) * DH],
                                tr_ps[:, h * 65 : h * 65 + 64],
                                rc[:],
                            )
                        nc.sync.dma_start(
                            out[n0 + sub * 128 : n0 + (sub + 1) * 128, :], o_sb[:]
                        )

            def emit_attention_headout():
                # head-outer: one head's full key sweep at a time. Only one PV
                # accumulator bank is live (+1 for overlap), so score tiles can
                # span 3 key-tiles (1536 fp32 = 3 PSUM banks, double-buffered):
                # 176 exp instructions/pass instead of 256, 176 mask-muls
                # instead of 512. Edge tiles are DMA'd once per chunk (h==0)
                # and re-read by the other 3 heads from SBUF.
                G = (MT + 2) // 3  # 22 tiles of up to 3 key-tiles each
                for rep in range(attn_reps):
                  for c in range(2):
                    n0 = c * 512
                    ctx = ctxpool.tile([128, H * 512], F32, tag="ctx")
                    # hoist the chunk's edge DMAs so h==0 mask-muls never wait
                    # on DMA issue order
                    etiles = []
                    for g in range(G):
                        t0 = 3 * g
                        tt = min(3, MT - t0)
                        e3 = epool.tile([128, 3, 512], BF16, tag="e")
                        nc.sync.dma_start(
                            e3[:, 0:tt, :],
                            edge[
                                t0 * 128 : (t0 + tt) * 128, n0 : n0 + 512
                            ].rearrange("(tt p) q -> p tt q", p=128),
                        )
                        etiles.append(e3)
                    for h in range(H):
                        pair, i = divmod(h, 2)
                        po = i * 64
                        pv = ps_pv.tile(
                            [128, 512], F32, tag="pv", name=f"pv_{rep}_{c}_{h}"
                        )

                        def emit_pv(t0, tt, p_sb):
                            for k in range(tt):
                                t = t0 + k
                                nc.tensor.matmul(
                                    pv[0:65, :],
                                    lhsT=Vp[
                                        :, t * VROW + h * 65 : t * VROW + h * 65 + 65
                                    ],
                                    rhs=p_sb[:, k, :],
                                    start=(t == 0),
                                    stop=(t == MT - 1),
                                )

                        # PV runs one tile behind QK/exp/mask: its mask is long
                        # done by then, so no PE instruction ever parks on a
                        # cross-engine wait (PE wait-queue depth is 4).
                        prev = None
                        for g in range(G):
                            t0 = 3 * g
                            tt = min(3, MT - t0)
                            e3 = etiles[g]
                            s_ps = ps_s.tile([128, 3, 512], F32, tag="sc")
                            for k in range(tt):
                                t = t0 + k
                                nc.tensor.matmul(
                                    s_ps[:, k, :],
                                    lhsT=KT[
                                        po : po + 64, pair, t * 128 : (t + 1) * 128
                                    ],
                                    rhs=QT[po : po + 64, pair, n0 : n0 + 512],
                                    start=True,
                                    stop=True,
                                )
                            p_sb = ppool.tile([128, 3, 512], BF16, tag="p")
                            # flatten to 2-D contiguous APs: engines only hit
                            # their fast path / documented rate on flat APs
                            pf = p_sb[:, 0:tt, :].rearrange("p a b -> p (a b)")
                            sf = s_ps[:, 0:tt, :].rearrange("p a b -> p (a b)")
                            ef = e3[:, 0:tt, :].rearrange("p a b -> p (a b)")
                            nc.scalar.activation(pf, sf, Exp, scale=0.125)
                            nc.vector.tensor_mul(pf, pf, ef)
                            if prev is not None:
                                emit_pv(*prev)
                            prev = (t0, tt, p_sb)
                        emit_pv(*prev)
                        nc.vector.tensor_copy(
                            ctx[0:65, h * 512 : (h + 1) * 512], pv[0:65, :]
                        )
                    # epilogue: divide by denominator row, transpose to [n, d]
                    for sub in range(4):
                        tr_ps = ps_s.tile([128, 3, 512], F32, tag="sc")
                        trf = tr_ps[:, :, :].rearrange("p a b -> p (a b)")
                        for h in range(H):
                            nc.tensor.transpose(
                                trf[:, h * 65 : h * 65 + 65],
                                ctx[
                                    0:65,
                                    h * 512 + sub * 128 : h * 512 + (sub + 1) * 128,
                                ],
                                id_sb[0:65, 0:65],
                            )
                        o_sb = opool.tile([128, D], F32, tag="o")
                        for h in range(H):
                            rc = rpool.tile([128, 1], F32, tag="rc")
                            nc.vector.reciprocal(
                                rc[:], trf[:, h * 65 + 64 : h * 65 + 65]
                            )
                            nc.vector.tensor_scalar_mul(
                                o_sb[:, h * DH : (h + 1) * DH],
                                trf[:, h * 65 : h * 65 + 64],
                                rc[:],
                            )
                        nc.sync.dma_start(
                            out[n0 + sub * 128 : n0 + (sub + 1) * 128, :], o_sb[:]
                        )

            emit = emit_attention_headout if ho else emit_attention
            if attn_loop is None:
                emit()
            else:
                with tc.For_i(0, attn_loop, 1):
                    emit()

    if split_drains:
        _split_drain_waits(nc)
    return nc


def prep_in_maps(x, edge, Wq, bq, Wk, bk, Wv, bv):
    bf16 = ml_dtypes.bfloat16
    x = np.ascontiguousarray(np.asarray(x, np.float32))
    edge = np.asarray(edge)
    xTr = np.ascontiguousarray(x.T.reshape(2, 128, N).transpose(1, 0, 2))

    def wprep(W):
        return np.ascontiguousarray(
            np.asarray(W, np.float32).reshape(2, 128, D).transpose(1, 0, 2)
        )

    def bprep(b):
        return np.ascontiguousarray(np.asarray(b, np.float32).reshape(2, 128).T)

    common = {
        "xTr": xTr,
        "wq": wprep(Wq),
        "wk": wprep(Wk),
        "wv": wprep(Wv),
        "bqc": bprep(bq),
        "bkc": bprep(bk),
        "bvb": np.ascontiguousarray(
            np.broadcast_to(np.asarray(bv, np.float32), (128, D))
        ),
        "ident": np.eye(128, dtype=np.float32),
        "identb": np.eye(128, dtype=np.float32).astype(bf16),
    }
    if VARIANT == "pemask":
        # additive mask: 0 where edge==1, -80000 (pre-exp-scale) where edge==0
        edge_bf = ((edge.astype(np.float32) - 1.0) * 80000.0).astype(bf16)
    else:
        edge_bf = edge.astype(bf16)
    in_maps = []
    for core in range(NCORES):
        n0 = core * NLOC
        m = dict(common)
        m["xq"] = np.ascontiguousarray(xTr[:, :, n0 : n0 + NLOC])
        m["edge"] = np.ascontiguousarray(edge_bf[:, n0 : n0 + NLOC])
        in_maps.append(m)
    return in_maps


_CACHED_NC = None


def kernel(x, edge, Wq, bq, Wk, bk, Wv, bv):
    global _CACHED_NC
    if _CACHED_NC is None:
        _CACHED_NC = build_module()
    nc = _CACHED_NC
    in_maps = prep_in_maps(x, edge, Wq, bq, Wk, bk, Wv, bv)
    res = bass_utils.run_bass_kernel_spmd(nc, in_maps, core_ids=list(range(NCORES)))
    out = np.concatenate([r["out"] for r in res.results], axis=0)
    return out.astype(np.float32)


if __name__ == "__main__":
    rng = np.random.default_rng(0)
    x = rng.standard_normal((N, D), dtype=np.float32)
    edge = rng.integers(0, 2, size=(N, N)).astype(np.int32)
    mk = lambda *s: (rng.standard_normal(s, dtype=np.float32) / 16.0)
    o = kernel(
        x, edge, mk(D, D), mk(D) * 0.16, mk(D, D), mk(D) * 0.16, mk(D, D), mk(D) * 0.16
    )
    print(o.shape, o.dtype)

